# revision 19
# baseline (speedup 1.0000x reference)
"""TRN2 Bass kernel for nn_EMAModule (EM attention module).

Computation (per sample):
    xf = conv1x1(x, w_in, b_in); T=3 EM iterations (softmax E-step over K=64
    bases, L2-normalized M-step); reconstruct; conv1x1(w_out, b_out);
    eval-BatchNorm; +residual.

Key algebraic restructuring (validated vs reference to ~3.6e-4 rel-to-scale):
    - xf is never materialized. The E-step logits are computed directly from x
      with a per-iteration folded weight  m2t = w_in^T-contract(mu)  (C,K small)
      and bias row beta_k = b_in . mu.
    - The M-step moment  sum_n z[n,k] xf[n,c]  is computed as
      G = Z^T X^T  (K,C)  then  mu_acc = G w_in^T + s b_in  (the epsilon-exact
      b_in term is added post-division; deviation ~1e-16).
    - The output path folds w_out, BatchNorm scale into  m3 = (w_out*inv) mu^T
      (K,O small), BN shift+bias into a row S appended via a 65th contraction
      row, and the residual via an identity-weight accumulating matmul.
    - All matmul operands are fp16 (PE runs fp16 at 1 cycle/row; exact given
      fp16 inputs). Statistics (softmax sums, norms) are fp32.

Sharding: data-parallel over batch, 2 samples per NeuronCore on 8 cores.
"""
import numpy as np

import concourse.bacc as bacc
import concourse.bass as bass
import concourse.tile as tile
from concourse import mybir
from concourse import bass_utils
from concourse.masks import make_identity

F32 = mybir.dt.float32
F16 = mybir.dt.float16
AF = mybir.ActivationFunctionType

B, C, H, W, K = 16, 512, 64, 64, 64
N = H * W                 # 4096
NCORES = 8
SPC = B // NCORES         # samples per core = 2
T = 3
BN_EPS = 1e-5
EXP_SHIFT = -6.0          # exp(logit + EXP_SHIFT): cancels in softmax ratio,
                          # keeps exp within fp16 range (logits observed <= 13.2)
CC = C // 128             # 4 channel chunks
NT = N // 128             # 32 n-tiles
NQ = 4                    # logits quarters
NTQ = NT // NQ            # 8 n-tiles per quarter
NK = N // 512             # 8 n-chunks of 512


def ts(i, sz):
    return bass.ts(i, sz)


def build_bass():
    nc = bacc.Bacc("TRN2", target_bir_lowering=False, debug=False,
                   num_devices=NCORES)
    dram = lambda name, shape, dt, kind: nc.dram_tensor(name, shape, dt, kind=kind).ap()
    x16 = dram("x16", [SPC, 128, CC, N], F16, "ExternalInput")
    xt16 = dram("xt16", [SPC, 128, NT, C], F16, "ExternalInput")
    # concatenated weights: [w_in (O,C) | w_in^T (C,O) | (w_out*inv)^T (C,O) |
    #                         bases^T (C,K) | b_in column], one DMA
    wcat = dram("wcat", [128, CC, 3 * C + K + 1], F16, "ExternalInput")
    srow16 = dram("srow16", [1, C], F16, "ExternalInput")      # BN shift row S
    binrow = dram("binrow", [1, C], F32, "ExternalInput")      # b_in row (fp32)
    outp = dram("outp", [SPC, CC, 128, N], F32, "ExternalOutput")

    with tile.TileContext(nc) as tc:
        with (
            tc.tile_pool(name="const", bufs=1) as cpool,
            tc.tile_pool(name="xin", bufs=2) as xpool,
            tc.tile_pool(name="xt", bufs=2) as xtpool,
            tc.tile_pool(name="work", bufs=2) as wpool,
            tc.tile_pool(name="outsb", bufs=3) as opool,
            tc.tile_pool(name="lg", bufs=2, space="PSUM") as lgpool,
            tc.tile_pool(name="sc", bufs=1, space="PSUM") as scpool,
            tc.tile_pool(name="srow", bufs=2, space="PSUM") as rowpool,
        ):
            # ---- constants ----
            wcat_sb = cpool.tile([128, CC, 3 * C + K + 1], F16)
            nc.sync.dma_start(out=wcat_sb, in_=wcat)
            w_sb = wcat_sb[:, :, 0:C]
            wt_sb = wcat_sb[:, :, C:2 * C]
            wot_sb = wcat_sb[:, :, 2 * C:3 * C]
            bt_sb = wcat_sb[:, :, 3 * C:3 * C + K]
            bin_sb = wcat_sb[:, :, 3 * C + K:3 * C + K + 1]
            binb_sb = cpool.tile([K, C], F32)          # b_in broadcast over K rows
            nc.sync.dma_start(out=binb_sb, in_=bass.AP(
                tensor=binrow.tensor, offset=binrow.offset,
                ap=[[0, K]] + binrow.ap[1:]))
            ident = cpool.tile([128, 128], F16)
            make_identity(nc, ident)
            ones_row = cpool.tile([1, 128], F16)
            nc.vector.memset(ones_row, 1.0)
            ones_col = cpool.tile([128, 1], F16)
            nc.vector.memset(ones_col, 1.0)
            expbias = cpool.tile([128, 1], F32)
            nc.vector.memset(expbias, EXP_SHIFT)
            ident32 = cpool.tile([1, 1], F32)
            nc.vector.memset(ident32, 1.0)

            # per-sample loads
            X, XT, muT, Z = [None] * SPC, [None] * SPC, [None] * SPC, [None] * SPC
            for s in range(SPC):
                X[s] = xpool.tile([128, CC, N], F16, tag="x", name=f"X{s}")
                for q in range(NQ):
                    nc.sync.dma_start(out=X[s][:, :, ts(q, N // NQ)],
                                      in_=x16[s][:, :, ts(q, N // NQ)])
                XT[s] = xtpool.tile([128, NT, C], F16, tag="xt", name=f"XT{s}")
                for q in range(NQ):
                    nc.sync.dma_start(out=XT[s][:, ts(q, NTQ), :],
                                      in_=xt16[s][:, ts(q, NTQ), :])
                muT[s] = bt_sb                  # (128, CC, K) fp16

            # EM iterations. Each iteration is emitted as phase A (PE-heavy
            # logit matmuls + per-quarter softmax head) for both samples, then
            # phase B (softmax tail + M-step) for both samples, so the static
            # per-engine schedules dovetail: while sample 0 is in a serial
            # DVE/ACT stretch, the PE runs sample 1's matmuls.
            E = [None] * SPC
            rinv = [None] * SPC
            for it in range(T):
                for s in range(SPC):
                    # ---- phase A: m2t, logits, exp, eb-scale, row sums
                    m2t_ps = scpool.tile([128, CC, K], F32, tag=f"sc{s}",
                                         name=f"m2t_ps{s}")
                    for cc in range(CC):
                        for oc in range(CC):
                            nc.tensor.matmul(
                                m2t_ps[:, cc, :],
                                w_sb[:, oc, ts(cc, 128)],
                                muT[s][:, oc, :],
                                start=(oc == 0), stop=(oc == CC - 1))
                    beta_ps = rowpool.tile([1, K], F32, tag="row",
                                           name=f"beta_ps{s}")
                    for oc in range(CC):
                        nc.tensor.matmul(beta_ps, bin_sb[:, oc, :], muT[s][:, oc, :],
                                         start=(oc == 0), stop=(oc == CC - 1))
                    m2t_sb = wpool.tile([128, CC, K], F16, tag="m2t",
                                        name=f"m2t_sb{s}")
                    nc.scalar.copy(m2t_sb, m2t_ps)
                    beta_sb = wpool.tile([1, K], F16, tag="beta",
                                         name=f"beta_sb{s}")
                    nc.scalar.copy(beta_sb, beta_ps)
                    # softmax bias enters multiplicatively: eb = exp(beta),
                    # replicated to all partitions via a 1-row outer product
                    eb_row = wpool.tile([1, K], F16, tag="eb_row",
                                        name=f"eb_row{s}")
                    nc.scalar.activation(eb_row, beta_sb, AF.Exp)
                    eb_ps = rowpool.tile([128, K], F32, tag="row",
                                         name=f"eb_ps{s}")
                    nc.tensor.matmul(eb_ps, ones_row, eb_row, start=True, stop=True)
                    eb_b = wpool.tile([128, K], F16, tag="eb_b", name=f"eb_b{s}")
                    nc.scalar.copy(eb_b, eb_ps)

                    E[s] = wpool.tile([128, NT, K], F16, tag="E", name=f"E{s}")
                    r = wpool.tile([128, NT], F32, tag="r", name=f"r{s}")
                    rv = wpool.tile([128, NT], F32, tag="rinv", name=f"rinv{s}")
                    Z[s] = wpool.tile([128, NT, K], F16, tag="Z", name=f"Z_{s}")
                    for q in range(NQ):
                        lg = lgpool.tile([128, NTQ, K], F32, tag=f"lg{s}",
                                         name=f"lg{s}_{q}")
                        for t8 in range(NTQ):
                            t = q * NTQ + t8
                            for cc in range(CC):
                                nc.tensor.matmul(
                                    lg[:, t8, :],
                                    X[s][:, cc, ts(t, 128)],
                                    m2t_sb[:, cc, :],
                                    start=(cc == 0), stop=(cc == CC - 1))
                        Eq = E[s][:, ts(q, NTQ), :]
                        nc.scalar.activation(Eq, lg, AF.Exp,
                                             bias=expbias, scale=1.0)
                        nc.vector.tensor_tensor(
                            out=Eq, in0=Eq,
                            in1=bass.AP(tensor=eb_b.tensor, offset=eb_b.offset,
                                        ap=[eb_b.ap[0], [0, NTQ], [1, K]]),
                            op=mybir.AluOpType.mult)
                        nc.vector.reduce_sum(r[:, ts(q, NTQ)], Eq,
                                             axis=mybir.AxisListType.X)
                        rq = rv[:, ts(q, NTQ)]
                        nc.vector.reciprocal(rq, r[:, ts(q, NTQ)])
                        nc.vector.tensor_tensor(
                            out=Z[s][:, ts(q, NTQ), :], in0=Eq,
                            in1=bass.AP(tensor=rv.tensor,
                                        offset=rq.offset,
                                        ap=[rq.ap[0], rq.ap[1], [0, K]]),
                            op=mybir.AluOpType.mult)

                for s in range(SPC):
                    # ---- phase B: M-step, mu update
                    # G = Z^T X^T (K, C); s row = Z col sums (1, K)
                    G_ps = scpool.tile([K, C], F32, tag=f"sc{s}", name=f"G_ps{s}")
                    for t in range(NT):
                        nc.tensor.matmul(G_ps, Z[s][:, t, :], XT[s][:, t, :],
                                         start=(t == 0), stop=(t == NT - 1))
                    s_ps = rowpool.tile([1, NTQ, K], F32, tag="row",
                                        name=f"s_ps{s}")
                    for g in range(NQ):
                        nc.tensor.matmul(
                            s_ps, ones_col,
                            Z[s][:, ts(g, NTQ), :].rearrange("p a b -> p (a b)"),
                            start=(g == 0), stop=(g == NQ - 1))
                    G_sb = wpool.tile([K, C], F16, tag="G", name=f"G_sb{s}")
                    nc.vector.tensor_copy(G_sb, G_ps)
                    s_sb = wpool.tile([1, K], F32, tag="s", name=f"s_sb{s}")
                    nc.vector.reduce_sum(
                        s_sb, bass.AP(tensor=s_ps.tensor, offset=s_ps.offset,
                                      ap=[s_ps.ap[0], [1, K], [K, NTQ]]),
                        axis=mybir.AxisListType.X)
                    # transpose G -> GT (C-chunks, K); s row -> column
                    GT_ps = scpool.tile([128, CC, K], F16, tag=f"sc{s}",
                                        name=f"GT_ps{s}")
                    for cc in range(CC):
                        nc.tensor.transpose(GT_ps[:, cc, :], G_sb[:, ts(cc, 128)],
                                            ident[0:K, 0:K])
                    GT_sb = wpool.tile([128, CC, K], F16, tag="GT",
                                       name=f"GT_sb{s}")
                    nc.scalar.copy(GT_sb, GT_ps)
                    scol_ps = rowpool.tile([K, 1], F32, tag="row",
                                           name=f"scol_ps{s}")
                    nc.tensor.transpose(scol_ps, s_sb, ident32[0:1, 0:1])
                    qinv = wpool.tile([K, 1], F32, tag="qinv", name=f"qinv{s}")
                    nc.vector.tensor_scalar(qinv, scol_ps, 1e-12, None,
                                            op0=mybir.AluOpType.add)
                    nc.vector.reciprocal(qinv, qinv)
                    # mu_acc = GT^T-contract wt  (K, O)
                    mu_ps = scpool.tile([K, C], F32, tag=f"sc{s}", name=f"mu_ps{s}")
                    for cc in range(CC):
                        nc.tensor.matmul(mu_ps, GT_sb[:, cc, :], wt_sb[:, cc, :],
                                         start=(cc == 0), stop=(cc == CC - 1))
                    mu1 = wpool.tile([K, C], F32, tag="mu1", name=f"mu1_{s}")
                    nc.vector.tensor_scalar(mu1, mu_ps, qinv, None,
                                            op0=mybir.AluOpType.mult)
                    nc.vector.tensor_tensor(out=mu1, in0=mu1, in1=binb_sb,
                                            op=mybir.AluOpType.add)
                    # rn = 1/||mu1||: DVE square+reduce then Quake rsqrt
                    # (bit-trick seed + 3 Newton steps; no ACT tables involved)
                    sq = wpool.tile([K, C], F32, tag="sq", name=f"sq{s}")
                    nc.vector.tensor_tensor(out=sq, in0=mu1, in1=mu1,
                                            op=mybir.AluOpType.mult)
                    n2 = wpool.tile([K, 1], F32, tag="n2", name=f"n2_{s}")
                    nc.vector.reduce_sum(n2, sq, axis=mybir.AxisListType.X)
                    yy = wpool.tile([K, 1], F32, tag="yy", name=f"yy{s}")
                    ti = wpool.tile([K, 1], mybir.dt.int32, tag="ti",
                                    name=f"ti{s}")
                    nc.vector.tensor_scalar(ti, n2.bitcast(mybir.dt.int32), 1,
                                            None,
                                            op0=mybir.AluOpType.logical_shift_right)
                    nc.vector.tensor_scalar(ti, ti, -1, None,
                                            op0=mybir.AluOpType.bitwise_xor)
                    nc.vector.tensor_scalar(yy.bitcast(mybir.dt.int32), ti,
                                            0x5f3759df + 1, None,
                                            op0=mybir.AluOpType.add)
                    tb = wpool.tile([K, 1], F32, tag="tb", name=f"tb{s}")
                    for _ in range(3):
                        nc.vector.tensor_tensor(out=tb, in0=yy, in1=yy,
                                                op=mybir.AluOpType.mult)
                        nc.vector.tensor_tensor(out=tb, in0=tb, in1=n2,
                                                op=mybir.AluOpType.mult)
                        nc.vector.tensor_scalar(tb, tb, -0.5, 1.5,
                                                op0=mybir.AluOpType.mult,
                                                op1=mybir.AluOpType.add)
                        nc.vector.tensor_tensor(out=yy, in0=yy, in1=tb,
                                                op=mybir.AluOpType.mult)
                    mu16 = wpool.tile([K, C], F16, tag="mu16", name=f"mu16_{s}")
                    nc.vector.tensor_scalar(mu16, mu1, yy, None,
                                            op0=mybir.AluOpType.mult)
                    # transpose mu -> muT (C-chunks, K)
                    muT_ps = scpool.tile([128, CC, K], F16, tag=f"sc{s}",
                                         name=f"muT_ps{s}")
                    for cc in range(CC):
                        nc.tensor.transpose(muT_ps[:, cc, :], mu16[:, ts(cc, 128)],
                                            ident[0:K, 0:K])
                    muT_new = wpool.tile([128, CC, K], F16, tag="muT",
                                         name=f"muT_new{s}")
                    nc.scalar.copy(muT_new, muT_ps)
                    muT[s] = muT_new

            # ---- output path ----
            for s in range(SPC):
                # m3 = muT^T-contract wot  (K, O); append S row -> (K+1, O)
                m3_ps = scpool.tile([K, C], F32, tag=f"sc{s}", name=f"m3_ps{s}")
                for cc in range(CC):
                    nc.tensor.matmul(m3_ps, muT[s][:, cc, :], wot_sb[:, cc, :],
                                     start=(cc == 0), stop=(cc == CC - 1))
                m3s = wpool.tile([K + 1, C], F16, tag="m3s")
                nc.vector.tensor_copy(m3s[0:K, :], m3_ps)
                nc.sync.dma_start(out=m3s[K:K + 1, :], in_=srow16)
                # Z^T (K+1, N) with ones row
                ZT = wpool.tile([K + 1, N], F16, tag="ZT")
                nc.vector.memset(ZT[K:K + 1, :], 1.0)
                for g in range(NT // 4):
                    zt_ps = scpool.tile([K, 4, 128], F16, tag=f"sc{s}", name=f"zt_ps{s}_{g}")
                    for j in range(4):
                        nc.tensor.transpose(zt_ps[:, j, :], Z[s][:, g * 4 + j, :],
                                            ident)
                    nc.vector.tensor_copy(
                        ZT[0:K, ts(g, 512)].rearrange("p (a b) -> p a b", a=4),
                        zt_ps)
                # out2 = m3s^T-contract [ZT; ones] + residual, per (oc, nk) tile
                for oc in range(CC):
                    for half in range(4):
                        osb = opool.tile([128, 1024], F32, tag="osb",
                                         name=f"osb{s}_{oc}_{half}")
                        for j in range(2):
                            nk = half * 2 + j
                            o2 = lgpool.tile([128, 512], F32, tag=f"lg{s}",
                                             name=f"o2_{s}_{oc}_{nk}")
                            nc.tensor.matmul(o2, m3s[:, ts(oc, 128)],
                                             ZT[:, ts(nk, 512)],
                                             start=True, stop=False)
                            nc.tensor.matmul(o2, ident, X[s][:, oc, ts(nk, 512)],
                                             start=False, stop=True)
                            if (oc + nk) % 2 == 0:
                                nc.scalar.copy(osb[:, ts(j, 512)], o2)
                            else:
                                nc.vector.tensor_copy(osb[:, ts(j, 512)], o2)
                        nc.sync.dma_start(
                            out=outp[s, oc, :, ts(half, 1024)], in_=osb)

    nc.compile()
    return nc


_NC_CACHE = None
_RUN_KWARGS: dict = {}   # extra kwargs for run_bass_kernel_spmd (e.g. trace=True)
_LAST_RESULTS = None     # BassKernelResults of the most recent run


def _get_nc():
    global _NC_CACHE
    if _NC_CACHE is None:
        _NC_CACHE = build_bass()
    return _NC_CACHE


def kernel(x, w_in, b_in, w_out, b_out, gamma, beta, running_mean, running_var,
           bases):
    x = np.asarray(x, np.float32)
    w_in = np.asarray(w_in, np.float32)
    b_in = np.asarray(b_in, np.float32)
    w_out = np.asarray(w_out, np.float32)
    b_out = np.asarray(b_out, np.float32)
    gamma = np.asarray(gamma, np.float32)
    beta = np.asarray(beta, np.float32)
    running_mean = np.asarray(running_mean, np.float32)
    running_var = np.asarray(running_var, np.float32)
    bases = np.asarray(bases, np.float32)

    inv = gamma / np.sqrt(running_var + BN_EPS)
    S = b_out * inv + beta - running_mean * inv
    wot = (w_out * inv[:, None]).T                      # (C, O)

    xr = x.reshape(B, C, N)
    x16 = np.ascontiguousarray(
        xr.reshape(B, CC, 128, N).transpose(0, 2, 1, 3)).astype(np.float16)
    xt16 = np.ascontiguousarray(
        xr.transpose(0, 2, 1).reshape(B, NT, 128, C).transpose(0, 2, 1, 3)
    ).astype(np.float16)

    chunk = lambda a, f: a.reshape(CC, 128, f).transpose(1, 0, 2)
    wcat = np.ascontiguousarray(np.concatenate([
        chunk(w_in, C), chunk(np.ascontiguousarray(w_in.T), C),
        chunk(np.ascontiguousarray(wot), C),
        chunk(np.ascontiguousarray(bases.T), K), chunk(b_in, 1),
    ], axis=2)).astype(np.float16)
    srow16 = S.reshape(1, C).astype(np.float16)
    binrow = b_in.reshape(1, C).astype(np.float32)

    in_maps = []
    for core in range(NCORES):
        sl = slice(core * SPC, (core + 1) * SPC)
        in_maps.append({
            "x16": x16[sl], "xt16": xt16[sl],
            "wcat": wcat, "srow16": srow16, "binrow": binrow,
        })

    nc = _get_nc()
    res = bass_utils.run_bass_kernel_spmd(nc, in_maps, core_ids=list(range(NCORES)),
                                          **_RUN_KWARGS)
    global _LAST_RESULTS
    _LAST_RESULTS = res
    out = np.empty((B, C, N), np.float32)
    for core in range(NCORES):
        o = res.results[core]["outp"]                   # (SPC, CC, 128, N)
        out[core * SPC:(core + 1) * SPC] = o.reshape(SPC, C, N)
    return out.reshape(B, C, H, W)


# revision 20
# speedup vs baseline: 1.0095x; 1.0095x over previous
"""TRN2 Bass kernel for nn_EMAModule (EM attention module).

Computation (per sample):
    xf = conv1x1(x, w_in, b_in); T=3 EM iterations (softmax E-step over K=64
    bases, L2-normalized M-step); reconstruct; conv1x1(w_out, b_out);
    eval-BatchNorm; +residual.

Key algebraic restructuring (validated vs reference to ~3.6e-4 rel-to-scale):
    - xf is never materialized. The E-step logits are computed directly from x
      with a per-iteration folded weight  m2t = w_in^T-contract(mu)  (C,K small)
      and bias row beta_k = b_in . mu.
    - The M-step moment  sum_n z[n,k] xf[n,c]  is computed as
      G = Z^T X^T  (K,C)  then  mu_acc = G w_in^T + s b_in  (the epsilon-exact
      b_in term is added post-division; deviation ~1e-16).
    - The output path folds w_out, BatchNorm scale into  m3 = (w_out*inv) mu^T
      (K,O small), BN shift+bias into a row S appended via a 65th contraction
      row, and the residual via an identity-weight accumulating matmul.
    - All matmul operands are fp16 (PE runs fp16 at 1 cycle/row; exact given
      fp16 inputs). Statistics (softmax sums, norms) are fp32.

Sharding: data-parallel over batch, 2 samples per NeuronCore on 8 cores.
"""
import numpy as np

import concourse.bacc as bacc
import concourse.bass as bass
import concourse.tile as tile
from concourse import mybir
from concourse import bass_utils
from concourse.masks import make_identity

F32 = mybir.dt.float32
F16 = mybir.dt.float16
AF = mybir.ActivationFunctionType

B, C, H, W, K = 16, 512, 64, 64, 64
N = H * W                 # 4096
NCORES = 8
SPC = B // NCORES         # samples per core = 2
T = 3
BN_EPS = 1e-5
EXP_SHIFT = -6.0          # exp(logit + EXP_SHIFT): cancels in softmax ratio,
                          # keeps exp within fp16 range (logits observed <= 13.2)
CC = C // 128             # 4 channel chunks
NT = N // 128             # 32 n-tiles
NQ = 4                    # logits quarters
NTQ = NT // NQ            # 8 n-tiles per quarter
NK = N // 512             # 8 n-chunks of 512


def ts(i, sz):
    return bass.ts(i, sz)


def build_bass():
    nc = bacc.Bacc("TRN2", target_bir_lowering=False, debug=False,
                   num_devices=NCORES)
    dram = lambda name, shape, dt, kind: nc.dram_tensor(name, shape, dt, kind=kind).ap()
    x16 = dram("x16", [SPC, 128, CC, N], F16, "ExternalInput")
    xt16 = dram("xt16", [SPC, 128, NT, C], F16, "ExternalInput")
    # concatenated weights: [w_in (O,C) | w_in^T (C,O) | (w_out*inv)^T (C,O) |
    #                         bases^T (C,K) | b_in column], one DMA
    wcat = dram("wcat", [128, CC, 3 * C + K + 1], F16, "ExternalInput")
    srow16 = dram("srow16", [1, C], F16, "ExternalInput")      # BN shift row S
    binrow = dram("binrow", [1, C], F32, "ExternalInput")      # b_in row (fp32)
    outp = dram("outp", [SPC, CC, 128, N], F32, "ExternalOutput")

    with tile.TileContext(nc) as tc:
        with (
            tc.tile_pool(name="const", bufs=1) as cpool,
            tc.tile_pool(name="xin", bufs=2) as xpool,
            tc.tile_pool(name="xt", bufs=2) as xtpool,
            tc.tile_pool(name="work", bufs=2) as wpool,
            tc.tile_pool(name="outsb", bufs=4) as opool,
            tc.tile_pool(name="lg", bufs=2, space="PSUM") as lgpool,
            tc.tile_pool(name="sc", bufs=1, space="PSUM") as scpool,
            tc.tile_pool(name="srow", bufs=2, space="PSUM") as rowpool,
        ):
            # ---- constants ----
            wcat_sb = cpool.tile([128, CC, 3 * C + K + 1], F16)
            nc.scalar.dma_start(out=wcat_sb, in_=wcat)
            w_sb = wcat_sb[:, :, 0:C]
            wt_sb = wcat_sb[:, :, C:2 * C]
            wot_sb = wcat_sb[:, :, 2 * C:3 * C]
            bt_sb = wcat_sb[:, :, 3 * C:3 * C + K]
            bin_sb = wcat_sb[:, :, 3 * C + K:3 * C + K + 1]
            binb_sb = cpool.tile([K, C], F32)          # b_in broadcast over K rows
            nc.sync.dma_start(out=binb_sb, in_=bass.AP(
                tensor=binrow.tensor, offset=binrow.offset,
                ap=[[0, K]] + binrow.ap[1:]))
            ident = cpool.tile([128, 128], F16)
            make_identity(nc, ident)
            ones_row = cpool.tile([1, 128], F16)
            nc.vector.memset(ones_row, 1.0)
            ones_col = cpool.tile([128, 1], F16)
            nc.vector.memset(ones_col, 1.0)
            expbias = cpool.tile([128, 1], F32)
            nc.vector.memset(expbias, EXP_SHIFT)
            ident32 = cpool.tile([1, 1], F32)
            nc.vector.memset(ident32, 1.0)

            # per-sample loads
            X, XT, muT, Z = [None] * SPC, [None] * SPC, [None] * SPC, [None] * SPC
            for s in range(SPC):
                X[s] = xpool.tile([128, CC, N], F16, tag="x", name=f"X{s}")
                for q in range(NQ):
                    nc.sync.dma_start(out=X[s][:, :, ts(q, N // NQ)],
                                      in_=x16[s][:, :, ts(q, N // NQ)])
                XT[s] = xtpool.tile([128, NT, C], F16, tag="xt", name=f"XT{s}")
                for q in range(NQ):
                    nc.sync.dma_start(out=XT[s][:, ts(q, NTQ), :],
                                      in_=xt16[s][:, ts(q, NTQ), :])
                muT[s] = bt_sb                  # (128, CC, K) fp16

            # EM iterations. Each iteration is emitted as phase A (PE-heavy
            # logit matmuls + per-quarter softmax head) for both samples, then
            # phase B (softmax tail + M-step) for both samples, so the static
            # per-engine schedules dovetail: while sample 0 is in a serial
            # DVE/ACT stretch, the PE runs sample 1's matmuls.
            E = [None] * SPC
            rinv = [None] * SPC
            for it in range(T):
                for s in range(SPC):
                    # ---- phase A: m2t, logits, exp, eb-scale, row sums
                    m2t_ps = scpool.tile([128, CC, K], F32, tag=f"sc{s}",
                                         name=f"m2t_ps{s}")
                    for cc in range(CC):
                        for oc in range(CC):
                            nc.tensor.matmul(
                                m2t_ps[:, cc, :],
                                w_sb[:, oc, ts(cc, 128)],
                                muT[s][:, oc, :],
                                start=(oc == 0), stop=(oc == CC - 1))
                    beta_ps = rowpool.tile([1, K], F32, tag="row",
                                           name=f"beta_ps{s}")
                    for oc in range(CC):
                        nc.tensor.matmul(beta_ps, bin_sb[:, oc, :], muT[s][:, oc, :],
                                         start=(oc == 0), stop=(oc == CC - 1))
                    m2t_sb = wpool.tile([128, CC, K], F16, tag="m2t",
                                        name=f"m2t_sb{s}")
                    nc.scalar.copy(m2t_sb, m2t_ps)
                    # beta replicated to (1, NTQ, K) so one matmul per logits
                    # quarter pre-loads the bias into PSUM
                    beta8 = wpool.tile([1, NTQ, K], F16, tag="beta",
                                       name=f"beta8_{s}")
                    nc.scalar.copy(
                        beta8, bass.AP(tensor=beta_ps.tensor, offset=beta_ps.offset,
                                       ap=[beta_ps.ap[0], [0, NTQ], [1, K]]))

                    E[s] = wpool.tile([128, NT, K], F16, tag="E", name=f"E{s}")
                    r = wpool.tile([128, NT], F32, tag="r", name=f"r{s}")
                    rv = wpool.tile([128, NT], F32, tag="rinv", name=f"rinv{s}")
                    Z[s] = wpool.tile([128, NT, K], F16, tag="Z", name=f"Z_{s}")
                    for q in range(NQ):
                        lg = lgpool.tile([128, NTQ, K], F32, tag=f"lg{s}",
                                         name=f"lg{s}_{q}")
                        nc.tensor.matmul(lg, ones_row, beta8,
                                         start=True, stop=False)
                        for t8 in range(NTQ):
                            t = q * NTQ + t8
                            last = (t8 == NTQ - 1)
                            for cc in range(CC):
                                nc.tensor.matmul(
                                    lg[:, t8, :],
                                    X[s][:, cc, ts(t, 128)],
                                    m2t_sb[:, cc, :],
                                    start=False,
                                    stop=(last and cc == CC - 1))
                        Eq = E[s][:, ts(q, NTQ), :]
                        nc.scalar.activation(Eq, lg, AF.Exp,
                                             bias=expbias, scale=1.0)
                        nc.vector.reduce_sum(r[:, ts(q, NTQ)], Eq,
                                             axis=mybir.AxisListType.X)
                        rq = rv[:, ts(q, NTQ)]
                        nc.vector.reciprocal(rq, r[:, ts(q, NTQ)])
                        nc.vector.tensor_tensor(
                            out=Z[s][:, ts(q, NTQ), :], in0=Eq,
                            in1=bass.AP(tensor=rv.tensor,
                                        offset=rq.offset,
                                        ap=[rq.ap[0], rq.ap[1], [0, K]]),
                            op=mybir.AluOpType.mult)

                for s in range(SPC):
                    # ---- phase B: M-step, mu update
                    # G = Z^T X^T (K, C); s row = Z col sums (1, K)
                    G_ps = scpool.tile([K, C], F32, tag=f"sc{s}", name=f"G_ps{s}")
                    for t in range(NT):
                        nc.tensor.matmul(G_ps, Z[s][:, t, :], XT[s][:, t, :],
                                         start=(t == 0), stop=(t == NT - 1))
                    s_ps = rowpool.tile([1, K], F32, tag="row", name=f"s_ps{s}")
                    for t in range(NT):
                        nc.tensor.matmul(s_ps, ones_col, Z[s][:, t, :],
                                         start=(t == 0), stop=(t == NT - 1))
                    G_sb = wpool.tile([K, C], F16, tag="G", name=f"G_sb{s}")
                    nc.vector.tensor_copy(G_sb, G_ps)
                    s_sb = wpool.tile([1, K], F32, tag="s", name=f"s_sb{s}")
                    nc.scalar.copy(s_sb, s_ps)
                    # transpose G -> GT (C-chunks, K); s row -> column
                    GT_ps = scpool.tile([128, CC, K], F16, tag=f"sc{s}",
                                        name=f"GT_ps{s}")
                    for cc in range(CC):
                        nc.tensor.transpose(GT_ps[:, cc, :], G_sb[:, ts(cc, 128)],
                                            ident[0:K, 0:K])
                    GT_sb = wpool.tile([128, CC, K], F16, tag="GT",
                                       name=f"GT_sb{s}")
                    nc.scalar.copy(GT_sb, GT_ps)
                    scol_ps = rowpool.tile([K, 1], F32, tag="row",
                                           name=f"scol_ps{s}")
                    nc.tensor.transpose(scol_ps, s_sb, ident32[0:1, 0:1])
                    qinv = wpool.tile([K, 1], F32, tag="qinv", name=f"qinv{s}")
                    nc.vector.tensor_scalar(qinv, scol_ps, 1e-12, None,
                                            op0=mybir.AluOpType.add)
                    nc.vector.reciprocal(qinv, qinv)
                    # mu_acc = GT^T-contract wt  (K, O)
                    mu_ps = scpool.tile([K, C], F32, tag=f"sc{s}", name=f"mu_ps{s}")
                    for cc in range(CC):
                        nc.tensor.matmul(mu_ps, GT_sb[:, cc, :], wt_sb[:, cc, :],
                                         start=(cc == 0), stop=(cc == CC - 1))
                    mu1 = wpool.tile([K, C], F32, tag="mu1", name=f"mu1_{s}")
                    nc.vector.tensor_scalar(mu1, mu_ps, qinv, None,
                                            op0=mybir.AluOpType.mult)
                    nc.vector.tensor_tensor(out=mu1, in0=mu1, in1=binb_sb,
                                            op=mybir.AluOpType.add)
                    # rn = 1/||mu1||: DVE square+reduce then Quake rsqrt
                    # (bit-trick seed + 3 Newton steps; no ACT tables involved)
                    sq = wpool.tile([K, C], F32, tag="sq", name=f"sq{s}")
                    nc.vector.tensor_tensor(out=sq, in0=mu1, in1=mu1,
                                            op=mybir.AluOpType.mult)
                    n2 = wpool.tile([K, 1], F32, tag="n2", name=f"n2_{s}")
                    nc.vector.reduce_sum(n2, sq, axis=mybir.AxisListType.X)
                    yy = wpool.tile([K, 1], F32, tag="yy", name=f"yy{s}")
                    ti = wpool.tile([K, 1], mybir.dt.int32, tag="ti",
                                    name=f"ti{s}")
                    nc.vector.tensor_scalar(ti, n2.bitcast(mybir.dt.int32), 1,
                                            None,
                                            op0=mybir.AluOpType.logical_shift_right)
                    nc.vector.tensor_scalar(ti, ti, -1, None,
                                            op0=mybir.AluOpType.bitwise_xor)
                    nc.vector.tensor_scalar(yy.bitcast(mybir.dt.int32), ti,
                                            0x5f3759df + 1, None,
                                            op0=mybir.AluOpType.add)
                    tb = wpool.tile([K, 1], F32, tag="tb", name=f"tb{s}")
                    for _ in range(3):
                        nc.vector.tensor_tensor(out=tb, in0=yy, in1=yy,
                                                op=mybir.AluOpType.mult)
                        nc.vector.tensor_tensor(out=tb, in0=tb, in1=n2,
                                                op=mybir.AluOpType.mult)
                        nc.vector.tensor_scalar(tb, tb, -0.5, 1.5,
                                                op0=mybir.AluOpType.mult,
                                                op1=mybir.AluOpType.add)
                        nc.vector.tensor_tensor(out=yy, in0=yy, in1=tb,
                                                op=mybir.AluOpType.mult)
                    mu16 = wpool.tile([K, C], F16, tag="mu16", name=f"mu16_{s}")
                    nc.vector.tensor_scalar(mu16, mu1, yy, None,
                                            op0=mybir.AluOpType.mult)
                    # transpose mu -> muT (C-chunks, K)
                    muT_ps = scpool.tile([128, CC, K], F16, tag=f"sc{s}",
                                         name=f"muT_ps{s}")
                    for cc in range(CC):
                        nc.tensor.transpose(muT_ps[:, cc, :], mu16[:, ts(cc, 128)],
                                            ident[0:K, 0:K])
                    muT_new = wpool.tile([128, CC, K], F16, tag="muT",
                                         name=f"muT_new{s}")
                    nc.scalar.copy(muT_new, muT_ps)
                    muT[s] = muT_new

            # ---- output path ----
            for s in range(SPC):
                # m3 = muT^T-contract wot  (K, O); append S row -> (K+1, O)
                m3_ps = scpool.tile([K, C], F32, tag=f"sc{s}", name=f"m3_ps{s}")
                for cc in range(CC):
                    nc.tensor.matmul(m3_ps, muT[s][:, cc, :], wot_sb[:, cc, :],
                                     start=(cc == 0), stop=(cc == CC - 1))
                m3s = wpool.tile([K + 1, C], F16, tag="m3s")
                nc.vector.tensor_copy(m3s[0:K, :], m3_ps)
                nc.sync.dma_start(out=m3s[K:K + 1, :], in_=srow16)
                # Z^T (K+1, N) with ones row
                ZT = wpool.tile([K + 1, N], F16, tag="ZT")
                nc.vector.memset(ZT[K:K + 1, :], 1.0)
                for g in range(NT // 4):
                    zt_ps = scpool.tile([K, 4, 128], F16, tag=f"sc{s}", name=f"zt_ps{s}_{g}")
                    for j in range(4):
                        nc.tensor.transpose(zt_ps[:, j, :], Z[s][:, g * 4 + j, :],
                                            ident)
                    nc.vector.tensor_copy(
                        ZT[0:K, ts(g, 512)].rearrange("p (a b) -> p a b", a=4),
                        zt_ps)
                # out2 = m3s^T-contract [ZT; ones] + residual, per (oc, nk) tile
                for oc in range(CC):
                    for nk in range(NK):
                        o2 = lgpool.tile([128, 512], F32, tag=f"lg{s}",
                                         name=f"o2_{s}_{oc}_{nk}")
                        nc.tensor.matmul(o2, m3s[:, ts(oc, 128)],
                                         ZT[:, ts(nk, 512)],
                                         start=True, stop=False)
                        nc.tensor.matmul(o2, ident, X[s][:, oc, ts(nk, 512)],
                                         start=False, stop=True)
                        osb = opool.tile([128, 512], F32, tag="osb",
                                         name=f"osb{s}_{oc}_{nk}")
                        if (oc + nk) % 2 == 0:
                            nc.scalar.copy(osb, o2)
                        else:
                            nc.vector.tensor_copy(osb, o2)
                        nc.sync.dma_start(out=outp[s, oc, :, ts(nk, 512)],
                                          in_=osb)

    nc.compile()
    return nc


_NC_CACHE = None
_RUN_KWARGS: dict = {}   # extra kwargs for run_bass_kernel_spmd (e.g. trace=True)
_LAST_RESULTS = None     # BassKernelResults of the most recent run


def _get_nc():
    global _NC_CACHE
    if _NC_CACHE is None:
        _NC_CACHE = build_bass()
    return _NC_CACHE


def kernel(x, w_in, b_in, w_out, b_out, gamma, beta, running_mean, running_var,
           bases):
    x = np.asarray(x, np.float32)
    w_in = np.asarray(w_in, np.float32)
    b_in = np.asarray(b_in, np.float32)
    w_out = np.asarray(w_out, np.float32)
    b_out = np.asarray(b_out, np.float32)
    gamma = np.asarray(gamma, np.float32)
    beta = np.asarray(beta, np.float32)
    running_mean = np.asarray(running_mean, np.float32)
    running_var = np.asarray(running_var, np.float32)
    bases = np.asarray(bases, np.float32)

    inv = gamma / np.sqrt(running_var + BN_EPS)
    S = b_out * inv + beta - running_mean * inv
    wot = (w_out * inv[:, None]).T                      # (C, O)

    xr = x.reshape(B, C, N)
    x16 = np.ascontiguousarray(
        xr.reshape(B, CC, 128, N).transpose(0, 2, 1, 3)).astype(np.float16)
    xt16 = np.ascontiguousarray(
        xr.transpose(0, 2, 1).reshape(B, NT, 128, C).transpose(0, 2, 1, 3)
    ).astype(np.float16)

    chunk = lambda a, f: a.reshape(CC, 128, f).transpose(1, 0, 2)
    wcat = np.ascontiguousarray(np.concatenate([
        chunk(w_in, C), chunk(np.ascontiguousarray(w_in.T), C),
        chunk(np.ascontiguousarray(wot), C),
        chunk(np.ascontiguousarray(bases.T), K), chunk(b_in, 1),
    ], axis=2)).astype(np.float16)
    srow16 = S.reshape(1, C).astype(np.float16)
    binrow = b_in.reshape(1, C).astype(np.float32)

    in_maps = []
    for core in range(NCORES):
        sl = slice(core * SPC, (core + 1) * SPC)
        in_maps.append({
            "x16": x16[sl], "xt16": xt16[sl],
            "wcat": wcat, "srow16": srow16, "binrow": binrow,
        })

    nc = _get_nc()
    res = bass_utils.run_bass_kernel_spmd(nc, in_maps, core_ids=list(range(NCORES)),
                                          **_RUN_KWARGS)
    global _LAST_RESULTS
    _LAST_RESULTS = res
    out = np.empty((B, C, N), np.float32)
    for core in range(NCORES):
        o = res.results[core]["outp"]                   # (SPC, CC, 128, N)
        out[core * SPC:(core + 1) * SPC] = o.reshape(SPC, C, N)
    return out.reshape(B, C, H, W)
